# revision 55
# baseline (speedup 1.0000x reference)
"""Trainium2 Bass kernel for the BaseHeads pairwise-tanh head.

Computes, for x:(B,S,H)=(2,128,768), R=4 heads:
    s = x @ w_src.T + b_src   -> (B,S,R,H)
    t = x @ w_tgt.T + b_tgt   -> (B,S,R,H)
    out[b,r,i,j] = sum_h tanh(s[b,i,r,h] + t[b,j,r,h]) * w_out[h]

Sharding: one (b, r) pair per NeuronCore (B*R == 8 == n_cores), no
collectives.

Algorithm: instead of materializing the (S,S,H) pairwise tensor and
running tanh over all of it on the scalar engine (ACT-bound, ~100us),
approximate
    tanh(x) ~= c0*x + sum_k b_k sin(k*pi*x/L),   k in {1,2,4}, L=4.5
on the argument distribution.  Every sine factorizes over s+t:
    sin(w(s+t)) = sin(ws)cos(wt) + cos(ws)sin(wt)
so each harmonic becomes TWO rank-768 matmul chains (contraction over
h) on the otherwise-idle PE, and the elementwise work shrinks from
S*S*H to S*H per side.  The linear term is rank-2 (matmuls against a
ones tile).  End-to-end rel err (validated vs reference, incl fp16
quantization at every step): ~4.1e-3, vs the 2e-2 gate.

HW Sin is only valid on [-pi, pi]; base args om1*arg stay inside
(om1*max|arg_side| ~ 2.6), and cos/higher harmonics come from
half-angle + Chebyshev-style product recurrences:
    C1 = 1-2*sin^2(x/2), C2 = 1-2*S1^2, S2 = S1*(2*C1),
    C4 = 2*C2^2-1,       S4 = S2*(2*C2)
with w_out and the series coefficients folded into the s-side product
chain and into host-precomputed per-partition column slabs (wk).

Per-core dataflow:
  PE  : 72 projection matmuls (fp16), warm-up fillers, then 48 term
        matmuls accumulating the (S,S) logits in one PSUM tile
  ACT : PSUM drains (Identity, s-side bias fused), Sin/Square bases
  DVE : recurrences (tensor_scalar/tensor_tensor, fp16 fast modes),
        linear-term mults, final PSUM drain
  Pool/SP/ACT: DMA issue spread over the 3 DMA-capable queues
"""

import math
import sys

if "/opt/trn_rl_repo" not in sys.path:
    sys.path.insert(0, "/opt/trn_rl_repo")

import numpy as np

B, S, H, R = 2, 128, 768, 4
KC = H // 128  # 6 h-chunks
N_CORES = 8

# tanh(x) ~= C0*x + B1 sin(w1 x) + B2 sin(2 w1 x) + B4 sin(4 w1 x),
# w1 = pi/L.  Weighted LSQ fit on [-L, L], gaussian weight sigma=0.95.
L_FIT = 4.5
OM1 = math.pi / L_FIT
C0 = 0.28760255455681455
B1 = 0.3375764123981222
B2 = 0.24858671693929105
B4 = 0.0424362041404059

F16 = np.float16
N_FILL = 4  # PE p-state warm-up fillers

_PROGRAM_CACHE = {}
LAST_RESULTS = None  # BassKernelResults of the most recent run (for test.py)


def _ensure_ntff_hook():
    """The agent image's `antenv` stub lacks `axon_hooks`, so boot()'s NTFF
    profile-hook install silently degrades and bass_utils crashes on import
    when BASS_TRACE=1.  Inject a functional stand-in (module + ctypes hook)
    only if the real module is absent."""
    import importlib

    try:
        importlib.import_module("antenv.axon_hooks")
        return
    except ImportError:
        pass
    import types

    try:
        import antenv
    except ImportError:
        return
    mod = types.ModuleType("antenv.axon_hooks")
    mod._hook = None

    def set_axon_ntff_profile_hook(h):
        mod._hook = h

    def get_axon_ntff_profile_hook():
        return mod._hook

    mod.set_axon_ntff_profile_hook = set_axon_ntff_profile_hook
    mod.get_axon_ntff_profile_hook = get_axon_ntff_profile_hook
    sys.modules["antenv.axon_hooks"] = mod
    antenv.axon_hooks = mod
    try:
        from trn_agent_boot.trn_boot import _ntff_profile_via_ctypes

        hook = _ntff_profile_via_ctypes("/opt/axon/libaxon_pjrt.so")
        if hook is not None:
            mod._hook = hook
    except Exception:
        pass


def _build_program(split=True):
    import concourse.bass as bass
    import concourse.mybir as mybir
    from concourse.tile import TileContext

    f32 = mybir.dt.float32
    f16 = mybir.dt.float16
    Sin = mybir.ActivationFunctionType.Sin
    Sq = mybir.ActivationFunctionType.Square
    Ident = mybir.ActivationFunctionType.Identity
    MULT = mybir.AluOpType.mult
    ADD = mybir.AluOpType.add

    nc = bass.Bass()

    # Inputs (per-core, host pre-transposed, fp16 except the bias).
    # xt : (128, 768)  [p, kc*128+i]        = x[b].T chunk layout
    # ws : (128, 4608) [p, m*768+kc*128+j]  = w_src_r.T slab layout
    # wt : (128, 4608) same for w_tgt_r.T
    # bc : (128, 6)    [p, m] = (b_src+b_tgt)[r*768+m*128+p]   (f32)
    # wk : (128, 3072) [p, q*768+m*128+i] = coef_q*w_out[m*128+p],
    #      q in {lin: c0, k1: b1, k2: b2, k4: b4}  (constant along i)
    xt_d = nc.dram_tensor("xt", [128, H], f16, kind="ExternalInput")
    ws_d = nc.dram_tensor("ws", [128, KC * H], f16, kind="ExternalInput")
    wt_d = nc.dram_tensor("wt", [128, KC * H], f16, kind="ExternalInput")
    bc_d = nc.dram_tensor("bc", [128, KC], f32, kind="ExternalInput")
    wk_d = nc.dram_tensor("wk", [128, 4 * KC], f32, kind="ExternalInput")
    out_d = nc.dram_tensor("o", [128, S], f32, kind="ExternalOutput")

    be2 = B2 / B1
    be42 = B4 / B2

    with TileContext(nc) as tc:
        with (
            tc.tile_pool(name="const", bufs=1) as cp,
            tc.tile_pool(name="psproj", bufs=4, space="PSUM") as pp,
            tc.tile_pool(name="psout", bufs=1, space="PSUM") as po,
        ):
            xt = cp.tile([128, H], f16, tag="xt")
            ws_t = cp.tile([128, KC * H], f16, tag="ws")
            wt_t = cp.tile([128, KC * H], f16, tag="wt")
            bc = cp.tile([128, KC], f32, tag="bc")
            wk = cp.tile([128, 4 * H], f16, tag="wk")
            wkc = cp.tile([128, 4 * KC], f32, tag="wkc")
            ones = cp.tile([128, 128], f16, tag="ones")
            # concatenated argument tile, quarters [sA | tA | sB | tB]
            # (A = chunks 0-2, B = chunks 3-5 of each side)
            arg = cp.tile([128, 2 * H], f32, tag="arg")
            out_sb = cp.tile([128, S], f32, tag="osb")

            # fat function tiles (both sides, both halves): (128, 1536)
            def ff(tag):
                return cp.tile([128, 2 * H], f16, tag=tag, name=tag)

            # 384-wide-per-half family packed as (128, 768): [A | B]
            def fg(tag):
                return cp.tile([128, H], f16, tag=tag, name=tag)

            S1f, hf = ff("S1f"), ff("hf")
            SS1, hh, C1, C2, C2q = ff("SS1"), ff("hh"), ff("C1"), ff("C2"), ff("C2q")
            lin = ff("lin")
            C4s_, C4t_, tc1p_ = fg("C4s_"), fg("C4t_"), fg("tc1p_")
            wS1_, wC1_ = fg("wS1_"), fg("wC1_")
            wS2_, wC2_ = fg("wS2_"), fg("wC2_")
            wS4_, wC4_ = fg("wS4_"), fg("wC4_")
            S2t_, S4t_ = fg("S2t_"), fg("S4t_")

            scratch = cp.tile([128, 512], f16, tag="scratch")

            wk_lin = wk[:, 0:H]
            wk_1 = wk[:, H : 2 * H]
            wk_2 = wk[:, 2 * H : 3 * H]
            wk_4 = wk[:, 3 * H : 4 * H]

            # ---- DMA in: per-chunk pieces interleaved over the 3
            # DMA-capable queues, s-side weights first, wk blocks timed
            # to land just before their consumers. ----
            def chunk(t_sb, t_d, m):
                return dict(out=t_sb[:, m * H : (m + 1) * H], in_=t_d[:, m * H : (m + 1) * H])

            nc.vector.memset(ones, 1.0)
            nc.vector.memset(scratch, 0.5)

            # Early dummy activation FIRST on the ACT queue: triggers the
            # activation-table load during the DMA phase instead of on the
            # first drain.
            junk_act = cp.tile([128, 128], f16, tag="jact")
            nc.scalar.activation(junk_act, ones, Sin, bias=0.0, scale=1.0)

            # The very first weight chunk (ws0) is split across two queues
            # as their FIRST pieces, so the PE's projection stream starts
            # ~2.5us earlier instead of idling on the first full chunk.
            nc.sync.dma_start(out=ws_t[:, 0:384], in_=ws_d[:, 0:384])
            nc.sync.dma_start(out=bc, in_=bc_d[:, :])
            nc.sync.dma_start(out=wkc, in_=wk_d[:, :])
            nc.sync.dma_start(**chunk(wt_t, wt_d, 0))
            nc.sync.dma_start(**chunk(ws_t, ws_d, 3))
            nc.sync.dma_start(**chunk(wt_t, wt_d, 3))

            nc.scalar.dma_start(out=xt, in_=xt_d[:, :])
            nc.scalar.dma_start(**chunk(ws_t, ws_d, 1))
            nc.scalar.dma_start(**chunk(wt_t, wt_d, 1))
            nc.scalar.dma_start(**chunk(ws_t, ws_d, 4))
            nc.scalar.dma_start(**chunk(wt_t, wt_d, 4))

            nc.gpsimd.dma_start(out=ws_t[:, 384:768], in_=ws_d[:, 384:768])
            nc.gpsimd.dma_start(**chunk(ws_t, ws_d, 2))
            nc.gpsimd.dma_start(**chunk(wt_t, wt_d, 2))
            nc.gpsimd.dma_start(**chunk(ws_t, ws_d, 5))
            nc.gpsimd.dma_start(**chunk(wt_t, wt_d, 5))

            # Expand the tiny per-chunk coefficient columns into the full
            # wk slabs on DVE's dead time -- DVE is idle until the first
            # recurrence ops, while ACT must start draining PSUM early.
            for q in range(4):
                nc.vector.tensor_copy(
                    wk[:, q * H : (q + 1) * H].rearrange("p (m i) -> p m i", m=KC),
                    wkc[:, q * KC : (q + 1) * KC].unsqueeze(2).broadcast_to((128, KC, 128)),
                )

            # ---- PE warm-up fillers (p-state ramp) while weights land --
            ps_junk = po.tile([1, 512], f32, tag="junk")
            for i in range(N_FILL):
                nc.tensor.matmul(
                    ps_junk, ones[:, 0:1], scratch[:, :],
                    start=True, stop=True, skip_group_check=True,
                )

            # ---- projections: s/t chunks interleaved on PE, drained into
            # the concatenated arg tile (all drains on ACT, bias fused on
            # the s side). ----
            V = nc.vector
            G = nc.gpsimd

            def arg_col(side, m):
                X, mloc = divmod(m, 3)
                return X * H + (384 if side == "t" else 0) + mloc * 128

            def proj_chunk(side, m):
                side_w = ws_t if side == "s" else wt_t
                ps = pp.tile([128, 128], f32, tag="pp", name=f"pp_{side}{m}")
                for kc in range(KC):
                    nc.tensor.matmul(
                        ps,
                        side_w[:, m * H + kc * 128 : m * H + (kc + 1) * 128],
                        xt[:, kc * 128 : (kc + 1) * 128],
                        start=(kc == 0),
                        stop=(kc == KC - 1),
                    )
                c = arg_col(side, m)
                nc.scalar.activation(
                    arg[:, c : c + 128], ps, Ident,
                    bias=(bc[:, m : m + 1] if side == "s" else 0.0), scale=1.0,
                )

            out_ps = po.tile([128, S], f32, tag="ops")
            mm_state = {"i": 0}
            N_TERM_MM = 8 * KC

            def term_mm(lhs_ap, rhs_ap):
                nc.tensor.matmul(
                    out_ps, lhs_ap, rhs_ap,
                    start=(mm_state["i"] == 0),
                    stop=(mm_state["i"] == N_TERM_MM - 1),
                )
                mm_state["i"] += 1

            def half_funcs(X):
                """Bases, recurrences and weighted mults for half X
                (chunks 3X..3X+2 of both sides)."""
                F = slice(X * H, (X + 1) * H)           # fat slice (s|t)
                Fs = slice(X * H, X * H + 384)          # s sub
                Ft = slice(X * H + 384, (X + 1) * H)    # t sub
                Gx = slice(X * 384, (X + 1) * 384)      # packed-half slice

                nc.scalar.activation(S1f[:, F], arg[:, F], Sin, bias=0.0, scale=OM1)
                nc.scalar.activation(hf[:, F], arg[:, F], Sin, bias=0.0, scale=OM1 / 2)

                if X == 1:
                    # nothing follows on the ACT queue for half B, so its
                    # squares run there, unclogging the DVE tail
                    nc.scalar.activation(hh[:, F], hf[:, F], Sq)
                    nc.scalar.activation(SS1[:, F], S1f[:, F], Sq)
                else:
                    V.tensor_tensor(SS1[:, F], S1f[:, F], S1f[:, F], op=MULT)
                    V.tensor_tensor(hh[:, F], hf[:, F], hf[:, F], op=MULT)
                V.tensor_scalar(C1[:, F], hh[:, F], -2.0, 1.0, MULT, ADD)
                V.tensor_scalar(C2[:, F], SS1[:, F], -2.0, 1.0, MULT, ADD)
                V.tensor_scalar(tc1p_[:, Gx], hh[:, Fs], -4.0 * be2, 2.0 * be2, MULT, ADD)
                V.tensor_tensor(wS1_[:, Gx], S1f[:, Fs], wk_1[:, Gx], op=MULT)
                V.tensor_tensor(wC1_[:, Gx], C1[:, Fs], wk_1[:, Gx], op=MULT)
                # ordered so the tiles consumed by the late term chains
                # (S2t, S4t, wC2) complete early and only wC4 trails; the
                # cos-side weighted mults measured faster on DVE than on
                # Pool's slow DSP path, despite the queue pressure
                V.tensor_tensor(S2t_[:, Gx], S1f[:, Ft], C1[:, Ft], op=MULT)
                V.tensor_tensor(S4t_[:, Gx], S2t_[:, Gx], C2[:, Ft], op=MULT)
                V.tensor_tensor(wC2_[:, Gx], C2[:, Fs], wk_2[:, Gx], op=MULT)
                V.tensor_tensor(C2q[:, F], C2[:, F], C2[:, F], op=MULT)
                V.tensor_scalar(C4s_[:, Gx], C2q[:, Fs], 2.0, -1.0, MULT, ADD)
                V.tensor_scalar(C4t_[:, Gx], C2q[:, Ft], 4.0 * be42, -2.0 * be42, MULT, ADD)
                V.tensor_tensor(wS2_[:, Gx], wS1_[:, Gx], tc1p_[:, Gx], op=MULT)
                V.tensor_tensor(wS4_[:, Gx], wS2_[:, Gx], C2[:, Fs], op=MULT)
                V.tensor_tensor(wC4_[:, Gx], C4s_[:, Gx], wk_4[:, Gx], op=MULT)
                # linear term: both sides at once, wk_lin half broadcast
                # over the (s, t) pair (middle dim stride 0 keeps the
                # packed innermost dim, so fast mode is preserved)
                G.tensor_tensor(
                    lin[:, F].rearrange("p (two c) -> p two c", two=2),
                    arg[:, F].rearrange("p (two c) -> p two c", two=2),
                    wk_lin[:, Gx].unsqueeze(1).broadcast_to((128, 2, 384)),
                    op=MULT,
                )

            def half_terms(X):
                # this half's 24 term matmuls, chain-major so the PE's
                # in-order queue never stalls on a late chain mid-group
                # (the Pool-produced lin tiles come last)
                def cg(mloc):
                    return slice(X * 384 + mloc * 128, X * 384 + (mloc + 1) * 128)

                def cs(mloc):
                    return slice(X * H + mloc * 128, X * H + (mloc + 1) * 128)

                def ct(mloc):
                    return slice(X * H + 384 + mloc * 128, X * H + 384 + (mloc + 1) * 128)

                for m in range(3):
                    term_mm(wC1_[:, cg(m)], S1f[:, ct(m)])
                for m in range(3):
                    term_mm(wS1_[:, cg(m)], C1[:, ct(m)])
                for m in range(3):
                    term_mm(wS2_[:, cg(m)], C2[:, ct(m)])
                for m in range(3):
                    term_mm(wC2_[:, cg(m)], S2t_[:, cg(m)])
                for m in range(3):
                    term_mm(wS4_[:, cg(m)], C4t_[:, cg(m)])
                for m in range(3):
                    term_mm(wC4_[:, cg(m)], S4t_[:, cg(m)])
                for m in range(3):
                    term_mm(lin[:, cs(m)], ones[:, :])
                for m in range(3):
                    term_mm(ones[:, :], lin[:, ct(m)])

            # Emission order gives each engine a stall-free queue:
            #   PE : projA, projB, termsA, termsB
            #   ACT: drainsA, sinsA, drainsB, sinsB
            #   DVE: chainA, chainB
            for m in range(3):
                proj_chunk("s", m)
                proj_chunk("t", m)
            half_funcs(0)
            for m in range(3, KC):
                proj_chunk("s", m)
                proj_chunk("t", m)
            half_terms(0)
            half_funcs(1)
            half_terms(1)

            nc.vector.tensor_copy(out_sb, out_ps)
            nc.sync.dma_start(out=out_d[:, :], in_=out_sb)

    if split:
        _split_multi_waits(nc, mybir)
    return nc


def _split_multi_waits(nc, mybir):
    """This walrus build allows at most ONE sync-wait per instruction.
    Legalize by hoisting all but one wait onto same-engine NoOps placed
    immediately before the offending instruction (the engine executes its
    queue in order, so waiting on the NoOps first is equivalent)."""
    k = 0
    for func in nc.m.functions:
        for blk in func.blocks:
            insts = list(blk.instructions)
            out = []
            changed = False
            for inst in insts:
                si = inst.sync_info
                waits = list(si.on_wait) if si is not None and si.on_wait else []
                if len(waits) > 1:
                    changed = True
                    for w in waits[:-1]:
                        nop = mybir.InstNoOp(
                            name=f"WSPLIT-{k}",
                            engine=inst.engine,
                            sync_info=mybir.SyncInfo(on_wait=[w], on_update=[]),
                            ins=[],
                            outs=[],
                        )
                        k += 1
                        out.append(nop)
                    si.on_wait = [waits[-1]]
                out.append(inst)
            if changed:
                blk.instructions = out


def _prep_inputs(input_hidden_state, w_src, b_src, w_tgt, b_tgt, w_out):
    """Build the 8 per-core input dicts (host-side transpose/cast)."""
    x = np.asarray(input_hidden_state, dtype=np.float32)
    w_src = np.asarray(w_src, dtype=np.float32)
    w_tgt = np.asarray(w_tgt, dtype=np.float32)
    b_sum = np.asarray(b_src, dtype=np.float32) + np.asarray(b_tgt, dtype=np.float32)
    w_out = np.asarray(w_out, dtype=np.float32)

    # wk coefficient columns [lin | k1 | k2 | k4], (128, 4*KC) f32 --
    # expanded to full slabs on-device.  k2/k4 carry the folded
    # angle-doubling factors (S2t=sin2t/2, S4t=sin4t/4 on device).
    wo_col = np.ascontiguousarray(w_out.reshape(KC, 128).T)  # (128, KC)
    wk_tile = np.ascontiguousarray(
        np.concatenate(
            [coef * wo_col for coef in (C0, B1, 2.0 * B2, 4.0 * B4)], axis=1
        )
    ).astype(np.float32)

    in_maps = []
    for core in range(N_CORES):
        b, r = divmod(core, R)
        xT = x[b].T  # (H, S)
        xt = np.ascontiguousarray(
            xT.reshape(KC, 128, S).transpose(1, 0, 2).reshape(128, H)
        ).astype(F16)

        wT_s = w_src[r * H : (r + 1) * H, :].T.reshape(KC, 128, KC, 128)
        ws = np.ascontiguousarray(
            wT_s.transpose(1, 2, 0, 3).reshape(128, KC * H)
        ).astype(F16)
        wT_t = w_tgt[r * H : (r + 1) * H, :].T.reshape(KC, 128, KC, 128)
        wt = np.ascontiguousarray(
            wT_t.transpose(1, 2, 0, 3).reshape(128, KC * H)
        ).astype(F16)

        bc = np.ascontiguousarray(
            b_sum[r * H : (r + 1) * H].reshape(KC, 128).T
        ).astype(np.float32)

        in_maps.append({"xt": xt, "ws": ws, "wt": wt, "bc": bc, "wk": wk_tile})
    return in_maps


def kernel(input_hidden_state, w_src, b_src, w_tgt, b_tgt, w_out):
    global LAST_RESULTS
    _ensure_ntff_hook()
    from concourse.bass_utils import run_bass_kernel_spmd

    if "prog" not in _PROGRAM_CACHE:
        _PROGRAM_CACHE["prog"] = _build_program()
    nc = _PROGRAM_CACHE["prog"]

    in_maps = _prep_inputs(
        input_hidden_state, w_src, b_src, w_tgt, b_tgt, w_out
    )
    res = run_bass_kernel_spmd(nc, in_maps, core_ids=list(range(N_CORES)))
    LAST_RESULTS = res

    out = np.empty((B, R, S, S), dtype=np.float32)
    for core in range(N_CORES):
        b, r = divmod(core, R)
        out[b, r] = np.asarray(res.results[core]["o"], dtype=np.float32)
    return out


# revision 58
# speedup vs baseline: 1.1480x; 1.1480x over previous
"""Trainium2 Bass kernel for the BaseHeads pairwise-tanh head.

Computes, for x:(B,S,H)=(2,128,768), R=4 heads:
    s = x @ w_src.T + b_src   -> (B,S,R,H)
    t = x @ w_tgt.T + b_tgt   -> (B,S,R,H)
    out[b,r,i,j] = sum_h tanh(s[b,i,r,h] + t[b,j,r,h]) * w_out[h]

Sharding: one (b, r) pair per NeuronCore (B*R == 8 == n_cores), no
collectives.

Algorithm: instead of materializing the (S,S,H) pairwise tensor and
running tanh over all of it on the scalar engine (ACT-bound, ~100us),
approximate
    tanh(x) ~= c0*x + sum_k b_k sin(k*pi*x/L),   k in {1,2,4}, L=4.5
on the argument distribution.  Every sine factorizes over s+t:
    sin(w(s+t)) = sin(ws)cos(wt) + cos(ws)sin(wt)
so each harmonic becomes TWO rank-768 matmul chains (contraction over
h) on the otherwise-idle PE, and the elementwise work shrinks from
S*S*H to S*H per side.  The linear term is rank-2 (matmuls against a
ones tile).  End-to-end rel err (validated vs reference, incl fp16
quantization at every step): ~4.1e-3, vs the 2e-2 gate.

HW Sin is only valid on [-pi, pi]; base args om1*arg stay inside
(om1*max|arg_side| ~ 2.6), and cos/higher harmonics come from
half-angle + Chebyshev-style product recurrences:
    C1 = 1-2*sin^2(x/2), C2 = 1-2*S1^2, S2 = S1*(2*C1),
    C4 = 2*C2^2-1,       S4 = S2*(2*C2)
with w_out and the series coefficients folded into the s-side product
chain and into host-precomputed per-partition column slabs (wk).

Per-core dataflow:
  PE  : 72 projection matmuls (fp16), warm-up fillers, then 48 term
        matmuls accumulating the (S,S) logits in one PSUM tile
  ACT : PSUM drains (Identity, s-side bias fused), Sin/Square bases
  DVE : recurrences (tensor_scalar/tensor_tensor, fp16 fast modes),
        linear-term mults, final PSUM drain
  Pool/SP/ACT: DMA issue spread over the 3 DMA-capable queues
"""

import math
import sys

if "/opt/trn_rl_repo" not in sys.path:
    sys.path.insert(0, "/opt/trn_rl_repo")

import numpy as np

B, S, H, R = 2, 128, 768, 4
KC = H // 128  # 6 h-chunks
N_CORES = 8

# tanh(x) ~= C0*x + B1 sin(w1 x) + B2 sin(2 w1 x) + B4 sin(4 w1 x),
# w1 = pi/L.  Weighted LSQ fit on [-L, L], gaussian weight sigma=0.95.
L_FIT = 4.5
OM1 = math.pi / L_FIT
C0 = 0.28760255455681455
B1 = 0.3375764123981222
B2 = 0.24858671693929105
B4 = 0.0424362041404059

F16 = np.float16
N_FILL = 4  # PE p-state warm-up fillers

_PROGRAM_CACHE = {}
LAST_RESULTS = None  # BassKernelResults of the most recent run (for test.py)


def _ensure_ntff_hook():
    """The agent image's `antenv` stub lacks `axon_hooks`, so boot()'s NTFF
    profile-hook install silently degrades and bass_utils crashes on import
    when BASS_TRACE=1.  Inject a functional stand-in (module + ctypes hook)
    only if the real module is absent."""
    import importlib

    try:
        importlib.import_module("antenv.axon_hooks")
        return
    except ImportError:
        pass
    import types

    try:
        import antenv
    except ImportError:
        return
    mod = types.ModuleType("antenv.axon_hooks")
    mod._hook = None

    def set_axon_ntff_profile_hook(h):
        mod._hook = h

    def get_axon_ntff_profile_hook():
        return mod._hook

    mod.set_axon_ntff_profile_hook = set_axon_ntff_profile_hook
    mod.get_axon_ntff_profile_hook = get_axon_ntff_profile_hook
    sys.modules["antenv.axon_hooks"] = mod
    antenv.axon_hooks = mod
    try:
        from trn_agent_boot.trn_boot import _ntff_profile_via_ctypes

        hook = _ntff_profile_via_ctypes("/opt/axon/libaxon_pjrt.so")
        if hook is not None:
            mod._hook = hook
    except Exception:
        pass


def _build_program(split=True):
    import concourse.bass as bass
    import concourse.mybir as mybir
    from concourse.tile import TileContext

    f32 = mybir.dt.float32
    f16 = mybir.dt.float16
    Sin = mybir.ActivationFunctionType.Sin
    Sq = mybir.ActivationFunctionType.Square
    Ident = mybir.ActivationFunctionType.Identity
    MULT = mybir.AluOpType.mult
    ADD = mybir.AluOpType.add

    nc = bass.Bass()

    # Inputs (per-core, host pre-transposed, fp16 except the bias).
    # xt : (128, 768)  [p, kc*128+i]        = x[b].T chunk layout
    # ws : (128, 4608) [p, m*768+kc*128+j]  = w_src_r.T slab layout
    # wt : (128, 4608) same for w_tgt_r.T
    # bc : (128, 6)    [p, m] = (b_src+b_tgt)[r*768+m*128+p]   (f32)
    # wk : (128, 3072) [p, q*768+m*128+i] = coef_q*w_out[m*128+p],
    #      q in {lin: c0, k1: b1, k2: b2, k4: b4}  (constant along i)
    xt_d = nc.dram_tensor("xt", [128, H], f16, kind="ExternalInput")
    ws_d = nc.dram_tensor("ws", [128, KC * H], f16, kind="ExternalInput")
    wt_d = nc.dram_tensor("wt", [128, KC * H], f16, kind="ExternalInput")
    bc_d = nc.dram_tensor("bc", [128, KC], f32, kind="ExternalInput")
    wk_d = nc.dram_tensor("wk", [128, 4 * KC], f32, kind="ExternalInput")
    out_d = nc.dram_tensor("o", [128, S], f32, kind="ExternalOutput")

    be2 = B2 / B1
    be42 = B4 / B2

    with TileContext(nc) as tc:
        with (
            tc.tile_pool(name="const", bufs=1) as cp,
            tc.tile_pool(name="psproj", bufs=4, space="PSUM") as pp,
            tc.tile_pool(name="psout", bufs=1, space="PSUM") as po,
        ):
            xt = cp.tile([128, H], f16, tag="xt")
            ws_t = cp.tile([128, KC * H], f16, tag="ws")
            wt_t = cp.tile([128, KC * H], f16, tag="wt")
            bc = cp.tile([128, KC], f32, tag="bc")
            wk = cp.tile([128, 4 * H], f16, tag="wk")
            wkc = cp.tile([128, 4 * KC], f32, tag="wkc")
            ones = cp.tile([128, 128], f16, tag="ones")
            # concatenated argument tile, quarters [sA | tA | sB | tB]
            # (A = chunks 0-2, B = chunks 3-5 of each side)
            arg = cp.tile([128, 2 * H], f32, tag="arg")
            out_sb = cp.tile([128, S], f32, tag="osb")

            # fat function tiles (both sides, both halves): (128, 1536)
            def ff(tag):
                return cp.tile([128, 2 * H], f16, tag=tag, name=tag)

            # 384-wide-per-half family packed as (128, 768): [A | B]
            def fg(tag):
                return cp.tile([128, H], f16, tag=tag, name=tag)

            S1f, hf = ff("S1f"), ff("hf")
            SS1, hh, C1, C2, C2q = ff("SS1"), ff("hh"), ff("C1"), ff("C2"), ff("C2q")
            lin = ff("lin")
            C4s_, C4t_, tc1p_ = fg("C4s_"), fg("C4t_"), fg("tc1p_")
            wS1_, wC1_ = fg("wS1_"), fg("wC1_")
            wS2_, wC2_ = fg("wS2_"), fg("wC2_")
            wS4_, wC4_ = fg("wS4_"), fg("wC4_")
            S2t_, S4t_ = fg("S2t_"), fg("S4t_")

            scratch = cp.tile([128, 512], f16, tag="scratch")

            wk_lin = wk[:, 0:H]
            wk_1 = wk[:, H : 2 * H]
            wk_2 = wk[:, 2 * H : 3 * H]
            wk_4 = wk[:, 3 * H : 4 * H]

            # ---- DMA in: per-chunk pieces interleaved over the 3
            # DMA-capable queues, s-side weights first, wk blocks timed
            # to land just before their consumers. ----
            def chunk(t_sb, t_d, m):
                return dict(out=t_sb[:, m * H : (m + 1) * H], in_=t_d[:, m * H : (m + 1) * H])

            nc.vector.memset(ones, 1.0)
            nc.vector.memset(scratch, 0.5)

            # Early dummy activation FIRST on the ACT queue: triggers the
            # activation-table load during the DMA phase instead of on the
            # first drain.
            junk_act = cp.tile([128, 128], f16, tag="jact")
            nc.scalar.activation(junk_act, ones, Sin, bias=0.0, scale=1.0)

            # The very first weight chunk (ws0) is split across two queues
            # as their FIRST pieces, so the PE's projection stream starts
            # ~2.5us earlier instead of idling on the first full chunk.
            nc.sync.dma_start(out=ws_t[:, 0:384], in_=ws_d[:, 0:384])
            nc.sync.dma_start(out=bc, in_=bc_d[:, :])
            nc.sync.dma_start(**chunk(wt_t, wt_d, 0))
            nc.sync.dma_start(**chunk(ws_t, ws_d, 3))
            nc.sync.dma_start(**chunk(wt_t, wt_d, 3))

            nc.scalar.dma_start(out=xt, in_=xt_d[:, :])
            nc.scalar.dma_start(**chunk(ws_t, ws_d, 1))
            nc.scalar.dma_start(**chunk(wt_t, wt_d, 1))
            nc.scalar.dma_start(**chunk(ws_t, ws_d, 4))
            nc.scalar.dma_start(**chunk(wt_t, wt_d, 4))

            nc.gpsimd.dma_start(out=ws_t[:, 384:768], in_=ws_d[:, 384:768])
            nc.gpsimd.dma_start(out=wkc, in_=wk_d[:, :])
            nc.gpsimd.dma_start(**chunk(ws_t, ws_d, 2))
            nc.gpsimd.dma_start(**chunk(wt_t, wt_d, 2))
            nc.gpsimd.dma_start(**chunk(ws_t, ws_d, 5))
            nc.gpsimd.dma_start(**chunk(wt_t, wt_d, 5))

            # Expand the tiny per-chunk coefficient columns into the full
            # wk slabs on ACT's dead time (saves 0.59MB of weight DMA).
            for q in range(4):
                nc.scalar.activation(
                    wk[:, q * H : (q + 1) * H].rearrange("p (m i) -> p m i", m=KC),
                    wkc[:, q * KC : (q + 1) * KC].unsqueeze(2).broadcast_to((128, KC, 128)),
                    Ident, bias=0.0, scale=1.0,
                )

            # ---- PE warm-up fillers (p-state ramp) while weights land --
            ps_junk = po.tile([1, 512], f32, tag="junk")
            for i in range(N_FILL):
                nc.tensor.matmul(
                    ps_junk, ones[:, 0:1], scratch[:, :],
                    start=True, stop=True, skip_group_check=True,
                )

            # ---- projections: s/t chunks interleaved on PE, drained into
            # the concatenated arg tile (all drains on ACT, bias fused on
            # the s side). ----
            V = nc.vector
            G = nc.gpsimd

            def arg_col(side, m):
                X, mloc = divmod(m, 3)
                return X * H + (384 if side == "t" else 0) + mloc * 128

            def proj_chunk(side, m):
                side_w = ws_t if side == "s" else wt_t
                ps = pp.tile([128, 128], f32, tag="pp", name=f"pp_{side}{m}")
                for kc in range(KC):
                    nc.tensor.matmul(
                        ps,
                        side_w[:, m * H + kc * 128 : m * H + (kc + 1) * 128],
                        xt[:, kc * 128 : (kc + 1) * 128],
                        start=(kc == 0),
                        stop=(kc == KC - 1),
                    )
                c = arg_col(side, m)
                nc.scalar.activation(
                    arg[:, c : c + 128], ps, Ident,
                    bias=(bc[:, m : m + 1] if side == "s" else 0.0), scale=1.0,
                )

            out_ps = po.tile([128, S], f32, tag="ops")
            mm_state = {"i": 0}
            N_TERM_MM = 8 * KC

            def term_mm(lhs_ap, rhs_ap):
                nc.tensor.matmul(
                    out_ps, lhs_ap, rhs_ap,
                    start=(mm_state["i"] == 0),
                    stop=(mm_state["i"] == N_TERM_MM - 1),
                )
                mm_state["i"] += 1

            def half_funcs(X):
                """Bases, recurrences and weighted mults for half X
                (chunks 3X..3X+2 of both sides)."""
                F = slice(X * H, (X + 1) * H)           # fat slice (s|t)
                Fs = slice(X * H, X * H + 384)          # s sub
                Ft = slice(X * H + 384, (X + 1) * H)    # t sub
                Gx = slice(X * 384, (X + 1) * 384)      # packed-half slice

                nc.scalar.activation(S1f[:, F], arg[:, F], Sin, bias=0.0, scale=OM1)
                nc.scalar.activation(hf[:, F], arg[:, F], Sin, bias=0.0, scale=OM1 / 2)

                if X == 1:
                    # nothing follows on the ACT queue for half B, so its
                    # squares run there, unclogging the DVE tail
                    nc.scalar.activation(hh[:, F], hf[:, F], Sq)
                    nc.scalar.activation(SS1[:, F], S1f[:, F], Sq)
                else:
                    V.tensor_tensor(SS1[:, F], S1f[:, F], S1f[:, F], op=MULT)
                    V.tensor_tensor(hh[:, F], hf[:, F], hf[:, F], op=MULT)
                V.tensor_scalar(C1[:, F], hh[:, F], -2.0, 1.0, MULT, ADD)
                V.tensor_scalar(C2[:, F], SS1[:, F], -2.0, 1.0, MULT, ADD)
                V.tensor_scalar(tc1p_[:, Gx], hh[:, Fs], -4.0 * be2, 2.0 * be2, MULT, ADD)
                V.tensor_tensor(wS1_[:, Gx], S1f[:, Fs], wk_1[:, Gx], op=MULT)
                V.tensor_tensor(wC1_[:, Gx], C1[:, Fs], wk_1[:, Gx], op=MULT)
                # ordered so the tiles consumed by the late term chains
                # (S2t, S4t, wC2) complete early and only wC4 trails; the
                # cos-side weighted mults measured faster on DVE than on
                # Pool's slow DSP path, despite the queue pressure
                V.tensor_tensor(S2t_[:, Gx], S1f[:, Ft], C1[:, Ft], op=MULT)
                V.tensor_tensor(S4t_[:, Gx], S2t_[:, Gx], C2[:, Ft], op=MULT)
                V.tensor_tensor(wC2_[:, Gx], C2[:, Fs], wk_2[:, Gx], op=MULT)
                V.tensor_tensor(C2q[:, F], C2[:, F], C2[:, F], op=MULT)
                V.tensor_scalar(C4s_[:, Gx], C2q[:, Fs], 2.0, -1.0, MULT, ADD)
                V.tensor_scalar(C4t_[:, Gx], C2q[:, Ft], 4.0 * be42, -2.0 * be42, MULT, ADD)
                V.tensor_tensor(wS2_[:, Gx], wS1_[:, Gx], tc1p_[:, Gx], op=MULT)
                V.tensor_tensor(wS4_[:, Gx], wS2_[:, Gx], C2[:, Fs], op=MULT)
                V.tensor_tensor(wC4_[:, Gx], C4s_[:, Gx], wk_4[:, Gx], op=MULT)
                # linear term: both sides at once, wk_lin half broadcast
                # over the (s, t) pair (middle dim stride 0 keeps the
                # packed innermost dim, so fast mode is preserved)
                G.tensor_tensor(
                    lin[:, F].rearrange("p (two c) -> p two c", two=2),
                    arg[:, F].rearrange("p (two c) -> p two c", two=2),
                    wk_lin[:, Gx].unsqueeze(1).broadcast_to((128, 2, 384)),
                    op=MULT,
                )

            def half_terms(X):
                # this half's 24 term matmuls, chain-major so the PE's
                # in-order queue never stalls on a late chain mid-group
                # (the Pool-produced lin tiles come last)
                def cg(mloc):
                    return slice(X * 384 + mloc * 128, X * 384 + (mloc + 1) * 128)

                def cs(mloc):
                    return slice(X * H + mloc * 128, X * H + (mloc + 1) * 128)

                def ct(mloc):
                    return slice(X * H + 384 + mloc * 128, X * H + 384 + (mloc + 1) * 128)

                for m in range(3):
                    term_mm(wC1_[:, cg(m)], S1f[:, ct(m)])
                for m in range(3):
                    term_mm(wS1_[:, cg(m)], C1[:, ct(m)])
                for m in range(3):
                    term_mm(wS2_[:, cg(m)], C2[:, ct(m)])
                for m in range(3):
                    term_mm(wC2_[:, cg(m)], S2t_[:, cg(m)])
                for m in range(3):
                    term_mm(wS4_[:, cg(m)], C4t_[:, cg(m)])
                for m in range(3):
                    term_mm(wC4_[:, cg(m)], S4t_[:, cg(m)])
                for m in range(3):
                    term_mm(lin[:, cs(m)], ones[:, :])
                for m in range(3):
                    term_mm(ones[:, :], lin[:, ct(m)])

            # Emission order gives each engine a stall-free queue:
            #   PE : projA, projB, termsA, termsB
            #   ACT: drainsA, sinsA, drainsB, sinsB
            #   DVE: chainA, chainB
            for m in range(3):
                proj_chunk("s", m)
                proj_chunk("t", m)
            half_funcs(0)
            for m in range(3, KC):
                proj_chunk("s", m)
                proj_chunk("t", m)
            half_terms(0)
            half_funcs(1)
            half_terms(1)

            nc.vector.tensor_copy(out_sb, out_ps)
            nc.sync.dma_start(out=out_d[:, :], in_=out_sb)

    if split:
        _split_multi_waits(nc, mybir)
    return nc


def _split_multi_waits(nc, mybir):
    """This walrus build allows at most ONE sync-wait per instruction.
    Legalize by hoisting all but one wait onto same-engine NoOps placed
    immediately before the offending instruction (the engine executes its
    queue in order, so waiting on the NoOps first is equivalent)."""
    k = 0
    for func in nc.m.functions:
        for blk in func.blocks:
            insts = list(blk.instructions)
            out = []
            changed = False
            for inst in insts:
                si = inst.sync_info
                waits = list(si.on_wait) if si is not None and si.on_wait else []
                if len(waits) > 1:
                    changed = True
                    for w in waits[:-1]:
                        nop = mybir.InstNoOp(
                            name=f"WSPLIT-{k}",
                            engine=inst.engine,
                            sync_info=mybir.SyncInfo(on_wait=[w], on_update=[]),
                            ins=[],
                            outs=[],
                        )
                        k += 1
                        out.append(nop)
                    si.on_wait = [waits[-1]]
                out.append(inst)
            if changed:
                blk.instructions = out


def _prep_inputs(input_hidden_state, w_src, b_src, w_tgt, b_tgt, w_out):
    """Build the 8 per-core input dicts (host-side transpose/cast)."""
    x = np.asarray(input_hidden_state, dtype=np.float32)
    w_src = np.asarray(w_src, dtype=np.float32)
    w_tgt = np.asarray(w_tgt, dtype=np.float32)
    b_sum = np.asarray(b_src, dtype=np.float32) + np.asarray(b_tgt, dtype=np.float32)
    w_out = np.asarray(w_out, dtype=np.float32)

    # wk coefficient columns [lin | k1 | k2 | k4], (128, 4*KC) f32 --
    # expanded to full slabs on-device.  k2/k4 carry the folded
    # angle-doubling factors (S2t=sin2t/2, S4t=sin4t/4 on device).
    wo_col = np.ascontiguousarray(w_out.reshape(KC, 128).T)  # (128, KC)
    wk_tile = np.ascontiguousarray(
        np.concatenate(
            [coef * wo_col for coef in (C0, B1, 2.0 * B2, 4.0 * B4)], axis=1
        )
    ).astype(np.float32)

    in_maps = []
    for core in range(N_CORES):
        b, r = divmod(core, R)
        xT = x[b].T  # (H, S)
        xt = np.ascontiguousarray(
            xT.reshape(KC, 128, S).transpose(1, 0, 2).reshape(128, H)
        ).astype(F16)

        wT_s = w_src[r * H : (r + 1) * H, :].T.reshape(KC, 128, KC, 128)
        ws = np.ascontiguousarray(
            wT_s.transpose(1, 2, 0, 3).reshape(128, KC * H)
        ).astype(F16)
        wT_t = w_tgt[r * H : (r + 1) * H, :].T.reshape(KC, 128, KC, 128)
        wt = np.ascontiguousarray(
            wT_t.transpose(1, 2, 0, 3).reshape(128, KC * H)
        ).astype(F16)

        bc = np.ascontiguousarray(
            b_sum[r * H : (r + 1) * H].reshape(KC, 128).T
        ).astype(np.float32)

        in_maps.append({"xt": xt, "ws": ws, "wt": wt, "bc": bc, "wk": wk_tile})
    return in_maps


def kernel(input_hidden_state, w_src, b_src, w_tgt, b_tgt, w_out):
    global LAST_RESULTS
    _ensure_ntff_hook()
    from concourse.bass_utils import run_bass_kernel_spmd

    if "prog" not in _PROGRAM_CACHE:
        _PROGRAM_CACHE["prog"] = _build_program()
    nc = _PROGRAM_CACHE["prog"]

    in_maps = _prep_inputs(
        input_hidden_state, w_src, b_src, w_tgt, b_tgt, w_out
    )
    res = run_bass_kernel_spmd(nc, in_maps, core_ids=list(range(N_CORES)))
    LAST_RESULTS = res

    out = np.empty((B, R, S, S), dtype=np.float32)
    for core in range(N_CORES):
        b, r = divmod(core, R)
        out[b, r] = np.asarray(res.results[core]["o"], dtype=np.float32)
    return out
